# revision 26
# baseline (speedup 1.0000x reference)
"""Trainium2 Bass kernel for DeiT-style attention + depthwise-conv block.

Computes, for x [N=32, L=577, C=768]:
  qkv = x @ w_qkv.T -> q,k,v (12 heads, hd=64)
  attn = softmax(q k^T / 8) @ v
  out  = attn (+ depthwise3x3(v) on patch tokens) @ w_proj.T + b_proj

Sharding: data-parallel over batch, 4 samples per core x 8 NeuronCores.

Layout/precision choices:
 - q,k projections run in fp8e4 (DoubleRow perf mode, 2 channel-tiles
   per matmul). Weights are pre-scaled by 32 on the host to stay in the
   fp8 normal range; the 1/1024 is folded into the softmax scale.
 - v is computed channel-major in bf16; the token-major copy for the PV
   matmul comes from PE transposes instead of a second full matmul.
 - Depthwise conv uses clipped-window scalar_tensor_tensor MACs on the
   vector engine.
 - Softmax denominator comes from a ones-column in the PV matmul.

Scheduling: instructions are emitted software-pipelined across samples.
While sample s runs its (Act-bound) attention heads, the PE queue is
fed sample s+1's projection matmuls and sample s-1's output projection,
keeping the tensor engine continuously busy (idle gaps reset the PE
p-state clock ramp, halving its effective rate).
"""
import sys

sys.path.insert(0, "/opt/trn_rl_repo")

import numpy as np

import concourse.bacc as bacc
import concourse.mybir as mybir
import concourse.tile as tile
from concourse.bass_utils import run_bass_kernel_spmd
from concourse.masks import make_identity


F32 = mybir.dt.float32
F32R = mybir.dt.float32r
BF16 = mybir.dt.bfloat16
F8E4 = mybir.dt.float8e4
U8 = mybir.dt.uint8
Exp = mybir.ActivationFunctionType.Exp
MULT = mybir.AluOpType.mult
ADD = mybir.AluOpType.add
DR = mybir.MatmulPerfMode.DoubleRow

N_CORES = 8
S = 4            # samples per core
C, L, H, HD = 768, 577, 12, 64
CT = C // 128    # 6 channel tiles
WSCALE = 32.0    # host-side fp8 weight scale for q,k
SCALE = (HD ** -0.5) / (WSCALE * WSCALE)
L_CHUNKS = [(i * 128, min(128, L - i * 128)) for i in range((L + 127) // 128)]
NLC = len(L_CHUNKS)
LN_SPLIT = [(0, 512), (512, 65)]
IMG = 24         # spatial side; L-1 == IMG*IMG
# qk m-tile emission order: head pair h needs tiles (h, 6+h)
QK_ORDER = [m for pair in zip(range(6), range(6, 12)) for m in pair]

_CACHE = {}
last_results = None  # BassKernelResults of the most recent run (for test harness)


def _build_nc():
    key = "full"
    if key in _CACHE:
        return _CACHE[key]
    nc = bacc.Bacc("TRN2", target_bir_lowering=False, debug=False,
                   num_devices=N_CORES)
    xT_d = nc.declare_dram_parameter("xT", [S, C, L], BF16, isOutput=False)
    x8_d = nc.declare_dram_parameter("x8", [S, 128, CT, L], U8, isOutput=False)
    wqk8_d = nc.declare_dram_parameter("wqk8", [128, CT, 2 * C], U8,
                                       isOutput=False)
    wvT_d = nc.declare_dram_parameter("wvT", [C, C], BF16, isOutput=False)
    wprojT_d = nc.declare_dram_parameter("wprojT", [C, C], F32, isOutput=False)
    wdwc_d = nc.declare_dram_parameter("wdwc", [C, 9], F32, isOutput=False)
    bdwc_d = nc.declare_dram_parameter("bdwc", [C, 1], F32, isOutput=False)
    bproj_d = nc.declare_dram_parameter("bproj", [1, C], F32, isOutput=False)
    y_d = nc.declare_dram_parameter("y", [S, L, C], F32, isOutput=True)

    with tile.TileContext(nc) as tc:
        with tc.tile_pool(name="wpool", bufs=1) as wpool, \
             tc.tile_pool(name="work", bufs=2) as work, \
             tc.tile_pool(name="mm", bufs=2, space="PSUM") as psum_mm, \
             tc.tile_pool(name="pv", bufs=2, space="PSUM") as psum_pv:

            # ---- resident weights (first-needed DMA'd first; wproj and
            # conv weights deferred until after the warm-up loads) ----
            wv = []
            for k in range(CT):
                t = wpool.tile([128, C], BF16, tag="wv", bufs=CT, name=f"wv{k}")
                nc.sync.dma_start(t[:], wvT_d[k * 128:(k + 1) * 128, :])
                wv.append(t)
            wqk8 = wpool.tile([128, CT, 2 * C], F8E4, tag="wqk8")
            nc.sync.dma_start(wqk8[:], wqk8_d[:].bitcast(F8E4))
            bproj_row = wpool.tile([1, C], F32, tag="bprow")
            nc.sync.dma_start(bproj_row[:], bproj_d[:])
            bproj_bc = wpool.tile([128, C], F32, tag="bpbc")
            nc.gpsimd.partition_broadcast(bproj_bc[:], bproj_row[:])
            ident = wpool.tile([128, 128], BF16, tag="ident")
            make_identity(nc, ident[:])
            wprojT = []
            wdwc = []
            bdwc = []

            def emit_late_weights():
                for k in range(CT):
                    t = wpool.tile([128, 9], F32, tag="wdwc", bufs=CT,
                                   name=f"wdwc{k}")
                    nc.sync.dma_start(t[:], wdwc_d[k * 128:(k + 1) * 128, :])
                    wdwc.append(t)
                    t = wpool.tile([128, 1], F32, tag="bdwc", bufs=CT,
                                   name=f"bdwc{k}")
                    nc.sync.dma_start(t[:], bdwc_d[k * 128:(k + 1) * 128, :])
                    bdwc.append(t)
                for k in range(CT):
                    t = wpool.tile([128, C], F32R, tag="wprojT", bufs=CT,
                                   name=f"wprojT{k}")
                    nc.sync.dma_start(t[:], wprojT_d[k * 128:(k + 1) * 128, :]
                                      .bitcast(F32R))
                    wprojT.append(t)

            # per-sample state, keyed s % 2 for double buffering
            state = {}

            def emit_load(s):
                xT = []
                for k in range(CT):
                    t = work.tile([128, L], BF16, tag="xT", bufs=2 * CT,
                                  name=f"xT{k}")
                    nc.sync.dma_start(t[:], xT_d[s, k * 128:(k + 1) * 128, :])
                    xT.append(t)
                x8 = work.tile([128, CT, L], F8E4, tag="x8", bufs=2, name="x8")
                nc.sync.dma_start(x8[:], x8_d[s].bitcast(F8E4))
                st = {"xT": xT, "x8": x8, "v_ch": [None] * CT,
                      "qk": [None] * 12, "v65": [None] * NLC,
                      "cacc": [None] * CT, "attn": [], "pv": [None] * H,
                      "rec": [None] * H}
                for ct in range(CT):
                    st["attn"].append(work.tile([128, L], F32R, tag="attn",
                                                bufs=2 * CT, name=f"attn{ct}"))
                state[s] = st
                return st

            def emit_vch(s, m):
                st = state[s]
                p = psum_mm.tile([128, 768], F32, tag="mm")
                for k in range(CT):
                    for (n0, nn) in LN_SPLIT:
                        nc.tensor.matmul(
                            p[:, n0:n0 + nn],
                            wv[k][:, m * 128:(m + 1) * 128],
                            st["xT"][k][:, n0:n0 + nn],
                            start=(k == 0), stop=(k == CT - 1))
                dst = work.tile([128, L], BF16, tag="vch", bufs=2 * CT,
                                name=f"vch{m}")
                if m % 2 == 0:
                    nc.vector.tensor_copy(dst[:], p[:, 0:L])
                else:
                    nc.scalar.copy(dst[:], p[:, 0:L])
                st["v_ch"][m] = dst

            def emit_conv(s, ct):
                st = state[s]
                eng = nc.vector
                v3 = st["v_ch"][ct][:, 1:L].rearrange("p (y x) -> p y x",
                                                      y=IMG, x=IMG)
                acc = work.tile([128, IMG * IMG], BF16, tag="cacc",
                                bufs=2 * CT, name=f"cacc{ct}")
                acc3 = acc[:].rearrange("p (y x) -> p y x", y=IMG, x=IMG)
                eng.tensor_scalar(out=acc3, in0=v3, scalar1=wdwc[ct][:, 4:5],
                                  scalar2=None, op0=MULT)
                for t in range(9):
                    if t == 4:
                        continue
                    dy, dx = t // 3, t % 3
                    oy0, oy1 = max(0, 1 - dy), IMG + min(0, 1 - dy)
                    ox0, ox1 = max(0, 1 - dx), IMG + min(0, 1 - dx)
                    iy0, ix0 = oy0 + dy - 1, ox0 + dx - 1
                    eng.scalar_tensor_tensor(
                        out=acc3[:, oy0:oy1, ox0:ox1],
                        in0=v3[:, iy0:iy0 + (oy1 - oy0), ix0:ix0 + (ox1 - ox0)],
                        scalar=wdwc[ct][:, t:t + 1],
                        in1=acc3[:, oy0:oy1, ox0:ox1],
                        op0=MULT, op1=ADD)
                st["cacc"][ct] = acc

            def emit_qk(s, m, copy_eng):
                st = state[s]
                p = psum_mm.tile([128, 768], F32, tag="mm")
                for kp in range(CT // 2):
                    w_ap = wqk8[:, 2 * kp:2 * kp + 2, m * 128:(m + 1) * 128]
                    for (n0, nn) in LN_SPLIT:
                        nc.tensor.matmul(
                            p[:, n0:n0 + nn],
                            w_ap,
                            st["x8"][:, 2 * kp:2 * kp + 2, n0:n0 + nn],
                            start=(kp == 0), stop=(kp == CT // 2 - 1),
                            perf_mode=DR)
                dst = work.tile([128, L], BF16, tag="qk", bufs=24,
                                name=f"qk{m}")
                if copy_eng == "act":
                    nc.scalar.copy(dst[:], p[:, 0:L])
                else:
                    nc.vector.tensor_copy(dst[:], p[:, 0:L])
                st["qk"][m] = dst

            def emit_v65(s, ci):
                st = state[s]
                (l0, lp) = L_CHUNKS[ci]
                t = work.tile([128, H * 65], BF16, tag="v65", bufs=2 * NLC,
                              name="v65t")
                t3 = t[:].rearrange("p (h w) -> p h w", h=H, w=65)
                nc.vector.memset(t3[0:lp, :, 64:65], 1.0)
                p = psum_mm.tile([128, 768], F32, tag="mm")
                pb = p[:].bitcast(BF16)
                for ct in range(CT):
                    nc.tensor.transpose(
                        pb[0:lp, ct * 128:(ct + 1) * 128],
                        st["v_ch"][ct][:, l0:l0 + lp],
                        ident[:])
                nc.vector.tensor_copy(
                    t3[0:lp, :, 0:64],
                    pb[0:lp, 0:768].rearrange("p (h w) -> p h w", h=H, w=64))
                st["v65"][ci] = t

            def emit_score_chunk(s, h, ci, expS):
                st = state[s]
                qt = st["qk"][h // 2]
                kt_ = st["qk"][6 + h // 2]
                hb = (h % 2) * 64
                (l0, lp) = L_CHUNKS[ci]
                p = psum_mm.tile([128, 768], F32, tag="mm")
                for (n0, nn) in LN_SPLIT:
                    nc.tensor.matmul(
                        p[0:lp, n0:n0 + nn],
                        kt_[hb:hb + 64, l0:l0 + lp],
                        qt[hb:hb + 64, n0:n0 + nn],
                        start=True, stop=True)
                e = work.tile([128, L], BF16, tag="expS", bufs=8,
                              name="expSt")
                nc.scalar.activation(e[0:lp, :], p[0:lp, 0:L], Exp,
                                     scale=SCALE)
                expS.append(e)

            def emit_pv(s, h, expS):
                st = state[s]
                pv = psum_pv.tile([128, L], F32, tag="pv")
                for ci, (l0, lp) in enumerate(L_CHUNKS):
                    for (n0, nn) in LN_SPLIT:
                        nc.tensor.matmul(
                            pv[0:65, n0:n0 + nn],
                            st["v65"][ci][0:lp, h * 65:(h + 1) * 65],
                            expS[ci][0:lp, n0:n0 + nn],
                            start=(ci == 0), stop=(ci == NLC - 1))
                sums = work.tile([1, L], F32, tag="sums", bufs=4,
                                 name="sums")
                nc.vector.tensor_copy(sums[:], pv[64:65, 0:L])
                rec = work.tile([1, L], F32, tag="rec", bufs=4, name="rec")
                nc.vector.reciprocal_approx_fast(out=rec[:], in_=sums[:])
                st["pv"][h] = pv
                st["rec"][h] = rec

            def emit_norm(s, h):
                st = state[s]
                ct = h // 2
                j = h % 2
                bc = work.tile([64, L], F32, tag="bc", bufs=3, name="bc")
                nc.gpsimd.partition_broadcast(bc[:], st["rec"][h][:])
                nc.vector.tensor_tensor(
                    out=st["attn"][ct][j * 64:(j + 1) * 64, :],
                    in0=st["pv"][h][0:64, :], in1=bc[:], op=MULT)
                st["pv"][h] = None
                if j == 1:
                    # attn[:, 1:] += conv_acc + b_dwc (once per channel tile)
                    nc.vector.scalar_tensor_tensor(
                        out=st["attn"][ct][:, 1:L], in0=st["cacc"][ct][:],
                        scalar=bdwc[ct][:, 0:1],
                        in1=st["attn"][ct][:, 1:L].bitcast(F32),
                        op0=ADD, op1=ADD)

            def emit_proj(s, ci):
                st = state[s]
                (l0, lp) = L_CHUNKS[ci]
                p = psum_mm.tile([128, 768], F32, tag="mm")
                for k in range(CT):
                    for (n0, nn) in ((0, 512), (512, 256)):
                        nc.tensor.matmul(
                            p[0:lp, n0:n0 + nn],
                            st["attn"][k][:, l0:l0 + lp],
                            wprojT[k][:, n0:n0 + nn],
                            start=(k == 0), stop=(k == CT - 1))
                ysb = work.tile([128, C], F32, tag="ysb", bufs=2)
                nc.vector.tensor_tensor(
                    out=ysb[0:lp, :], in0=p[0:lp, :], in1=bproj_bc[0:lp, :],
                    op=ADD)
                nc.sync.dma_start(y_d[s, l0:l0 + lp, :], ysb[0:lp, :])

            def pre_pieces(s):
                """Generator of prep-work pieces for sample s (PE-heavy,
                interleaved into the previous sample's attention)."""
                emit_load(s)
                # first head pair's q,k as early as possible
                emit_qk(s, QK_ORDER[0], "act")
                emit_qk(s, QK_ORDER[1], "act")
                yield
                for m in range(CT):
                    emit_vch(s, m)
                    yield
                for m in QK_ORDER[2:]:
                    emit_qk(s, m, "act")
                    yield
                for ci in range(NLC):
                    emit_v65(s, ci)
                    yield

            def attn_heads(s):
                for h in range(H):
                    if h % 2 == 0:
                        emit_conv(s, h // 2)
                    expS = []
                    for ci in range(NLC):
                        emit_score_chunk(s, h, ci, expS)
                        yield
                    emit_pv(s, h, expS)
                    emit_norm(s, h)
                    yield

            # ---- software-pipelined emission ----
            # warm-up: all prep for sample 0, then the late weight DMAs
            for _ in pre_pieces(0):
                pass
            emit_late_weights()
            for s in range(S):
                interleave = []
                if s + 1 < S:
                    interleave.append(pre_pieces(s + 1))
                if s > 0:
                    def proj_pieces(ps):
                        for ci in range(NLC):
                            emit_proj(ps, ci)
                            yield
                        del state[ps]
                    interleave.append(proj_pieces(s - 1))
                heads = attn_heads(s)
                # round-robin: one head, then ~3 interleave pieces
                queues = [heads] + interleave
                done = [False] * len(queues)
                weights = [6, 1, 1]  # heads : pre : proj per round
                while not all(done):
                    for qi, q in enumerate(queues):
                        if done[qi]:
                            continue
                        w = weights[qi] if qi < len(weights) else 1
                        for _ in range(w):
                            try:
                                next(q)
                            except StopIteration:
                                done[qi] = True
                                break
            # drain: proj of last sample
            for ci in range(NLC):
                emit_proj(S - 1, ci)

    nc.compile()
    _CACHE[key] = nc
    return nc


def make_in_maps(x, w_qkv, w_proj, b_proj, w_dwc, b_dwc):
    import ml_dtypes
    F8NP = ml_dtypes.float8_e4m3
    x = np.asarray(x, dtype=np.float32)
    N = x.shape[0]
    assert N == N_CORES * S
    # q,k weights, fp8, x32, laid out [p, ct, m] for DoubleRow lhsT
    wqk = np.asarray(w_qkv, np.float32)[:2 * C] * WSCALE        # [2C, C]
    wqk8 = np.ascontiguousarray(
        wqk.T.reshape(CT, 128, 2 * C).transpose(1, 0, 2).astype(F8NP))
    wvT = np.ascontiguousarray(
        np.asarray(w_qkv, np.float32)[2 * C:].T.astype(ml_dtypes.bfloat16))
    wprojT = np.ascontiguousarray(np.asarray(w_proj, np.float32).T)    # [C, C]
    wdwc9 = np.ascontiguousarray(np.asarray(w_dwc, np.float32).reshape(C, 9))
    bdwc = np.ascontiguousarray(np.asarray(b_dwc, np.float32).reshape(C, 1))
    bproj = np.ascontiguousarray(np.asarray(b_proj, np.float32).reshape(1, C))

    in_maps = []
    for i in range(N_CORES):
        xs = x[i * S:(i + 1) * S]                       # [S, L, C]
        xT = np.ascontiguousarray(
            xs.transpose(0, 2, 1).astype(ml_dtypes.bfloat16))  # [S, C, L]
        x8 = np.ascontiguousarray(
            xs.transpose(0, 2, 1).reshape(S, CT, 128, L)
            .transpose(0, 2, 1, 3).astype(F8NP))               # [S, 128, CT, L]
        in_maps.append({"xT": xT, "x8": x8.view(np.uint8),
                        "wqk8": wqk8.view(np.uint8), "wvT": wvT,
                        "wprojT": wprojT, "wdwc": wdwc9, "bdwc": bdwc,
                        "bproj": bproj})
    return in_maps


def kernel(x, w_qkv, w_proj, b_proj, w_dwc, b_dwc):
    global last_results
    nc = _build_nc()
    in_maps = make_in_maps(x, w_qkv, w_proj, b_proj, w_dwc, b_dwc)
    last_results = run_bass_kernel_spmd(nc, in_maps, list(range(N_CORES)))
    y = np.concatenate([r["y"] for r in last_results.results], axis=0)
    return y.astype(np.float32)


# revision 28
# speedup vs baseline: 1.0281x; 1.0281x over previous
"""Trainium2 Bass kernel for DeiT-style attention + depthwise-conv block.

Computes, for x [N=32, L=577, C=768]:
  qkv = x @ w_qkv.T -> q,k,v (12 heads, hd=64)
  attn = softmax(q k^T / 8) @ v
  out  = attn (+ depthwise3x3(v) on patch tokens) @ w_proj.T + b_proj

Sharding: data-parallel over batch, 4 samples per core x 8 NeuronCores.

Layout/precision choices:
 - q,k projections run in fp8e4 (DoubleRow perf mode, 2 channel-tiles
   per matmul). Weights are pre-scaled by 32 on the host to stay in the
   fp8 normal range; the 1/1024 is folded into the softmax scale.
 - v is computed channel-major in bf16; the token-major copy for the PV
   matmul comes from PE transposes instead of a second full matmul.
 - Depthwise conv uses clipped-window scalar_tensor_tensor MACs on the
   vector engine.
 - Softmax denominator comes from a ones-column in the PV matmul.

Scheduling: instructions are emitted software-pipelined across samples.
While sample s runs its (Act-bound) attention heads, the PE queue is
fed sample s+1's projection matmuls and sample s-1's output projection,
keeping the tensor engine continuously busy (idle gaps reset the PE
p-state clock ramp, halving its effective rate).
"""
import sys

sys.path.insert(0, "/opt/trn_rl_repo")

import numpy as np

import concourse.bacc as bacc
import concourse.mybir as mybir
import concourse.tile as tile
from concourse.bass_utils import run_bass_kernel_spmd
from concourse.masks import make_identity


F32 = mybir.dt.float32
F32R = mybir.dt.float32r
BF16 = mybir.dt.bfloat16
F8E4 = mybir.dt.float8e4
U8 = mybir.dt.uint8
Exp = mybir.ActivationFunctionType.Exp
MULT = mybir.AluOpType.mult
ADD = mybir.AluOpType.add
DR = mybir.MatmulPerfMode.DoubleRow

N_CORES = 8
S = 4            # samples per core
C, L, H, HD = 768, 577, 12, 64
CT = C // 128    # 6 channel tiles
WSCALE = 32.0    # host-side fp8 weight scale for q,k
SCALE = (HD ** -0.5) / (WSCALE * WSCALE)
L_CHUNKS = [(i * 128, min(128, L - i * 128)) for i in range((L + 127) // 128)]
NLC = len(L_CHUNKS)
LN_SPLIT = [(0, 512), (512, 65)]
IMG = 24         # spatial side; L-1 == IMG*IMG
# qk m-tile emission order: head pair h needs tiles (h, 6+h)
QK_ORDER = [m for pair in zip(range(6), range(6, 12)) for m in pair]

_CACHE = {}
last_results = None  # BassKernelResults of the most recent run (for test harness)


def _build_nc():
    key = "full"
    if key in _CACHE:
        return _CACHE[key]
    nc = bacc.Bacc("TRN2", target_bir_lowering=False, debug=False,
                   num_devices=N_CORES)
    xT_d = nc.declare_dram_parameter("xT", [S, C, L], BF16, isOutput=False)
    x8_d = nc.declare_dram_parameter("x8", [S, 128, CT, L], U8, isOutput=False)
    wqk8_d = nc.declare_dram_parameter("wqk8", [128, CT, 2 * C], U8,
                                       isOutput=False)
    wvT_d = nc.declare_dram_parameter("wvT", [C, C], BF16, isOutput=False)
    wprojT_d = nc.declare_dram_parameter("wprojT", [C, C], F32, isOutput=False)
    wdwc_d = nc.declare_dram_parameter("wdwc", [C, 9], F32, isOutput=False)
    bdwc_d = nc.declare_dram_parameter("bdwc", [C, 1], F32, isOutput=False)
    bproj_d = nc.declare_dram_parameter("bproj", [1, C], F32, isOutput=False)
    y_d = nc.declare_dram_parameter("y", [S, L, C], F32, isOutput=True)

    with tile.TileContext(nc) as tc:
        with tc.tile_pool(name="wpool", bufs=1) as wpool, \
             tc.tile_pool(name="work", bufs=2) as work, \
             tc.tile_pool(name="mm", bufs=2, space="PSUM") as psum_mm, \
             tc.tile_pool(name="pv", bufs=2, space="PSUM") as psum_pv:

            # ---- resident weights (first-needed DMA'd first; wproj and
            # conv weights deferred until after the warm-up loads) ----
            wv = []
            for k in range(CT):
                t = wpool.tile([128, C], BF16, tag="wv", bufs=CT, name=f"wv{k}")
                nc.sync.dma_start(t[:], wvT_d[k * 128:(k + 1) * 128, :])
                wv.append(t)
            wqk8 = wpool.tile([128, CT, 2 * C], F8E4, tag="wqk8")
            nc.sync.dma_start(wqk8[:], wqk8_d[:].bitcast(F8E4))
            bproj_row = wpool.tile([1, C], F32, tag="bprow")
            nc.sync.dma_start(bproj_row[:], bproj_d[:])
            bproj_bc = wpool.tile([128, C], F32, tag="bpbc")
            nc.gpsimd.partition_broadcast(bproj_bc[:], bproj_row[:])
            ident = wpool.tile([128, 128], BF16, tag="ident")
            make_identity(nc, ident[:])
            wprojT = []
            wdwc = []
            bdwc = []

            def emit_late_weights():
                for k in range(CT):
                    t = wpool.tile([128, 9], F32, tag="wdwc", bufs=CT,
                                   name=f"wdwc{k}")
                    nc.sync.dma_start(t[:], wdwc_d[k * 128:(k + 1) * 128, :])
                    wdwc.append(t)
                    t = wpool.tile([128, 1], F32, tag="bdwc", bufs=CT,
                                   name=f"bdwc{k}")
                    nc.sync.dma_start(t[:], bdwc_d[k * 128:(k + 1) * 128, :])
                    bdwc.append(t)
                for k in range(CT):
                    t = wpool.tile([128, C], F32R, tag="wprojT", bufs=CT,
                                   name=f"wprojT{k}")
                    nc.sync.dma_start(t[:], wprojT_d[k * 128:(k + 1) * 128, :]
                                      .bitcast(F32R))
                    wprojT.append(t)

            # per-sample state, keyed s % 2 for double buffering
            state = {}

            def emit_load(s):
                xT = []
                for k in range(CT):
                    t = work.tile([128, L], BF16, tag="xT", bufs=2 * CT,
                                  name=f"xT{k}")
                    nc.sync.dma_start(t[:], xT_d[s, k * 128:(k + 1) * 128, :])
                    xT.append(t)
                x8 = work.tile([128, CT, L], F8E4, tag="x8", bufs=2, name="x8")
                nc.sync.dma_start(x8[:], x8_d[s].bitcast(F8E4))
                st = {"xT": xT, "x8": x8, "v_ch": [None] * CT,
                      "qk": [None] * 12, "v65": [None] * NLC,
                      "cacc": [None] * CT, "attn": [], "pv": [None] * H,
                      "rec": [None] * H}
                for ct in range(CT):
                    st["attn"].append(work.tile([128, L], F32R, tag="attn",
                                                bufs=2 * CT, name=f"attn{ct}"))
                state[s] = st
                return st

            def emit_vch(s, m):
                st = state[s]
                p = psum_mm.tile([128, 768], F32, tag="mm")
                for k in range(CT):
                    for (n0, nn) in LN_SPLIT:
                        nc.tensor.matmul(
                            p[:, n0:n0 + nn],
                            wv[k][:, m * 128:(m + 1) * 128],
                            st["xT"][k][:, n0:n0 + nn],
                            start=(k == 0), stop=(k == CT - 1))
                dst = work.tile([128, L], BF16, tag="vch", bufs=2 * CT,
                                name=f"vch{m}")
                if m % 2 == 0:
                    nc.vector.tensor_copy(dst[:], p[:, 0:L])
                else:
                    nc.scalar.copy(dst[:], p[:, 0:L])
                st["v_ch"][m] = dst

            def emit_conv(s, ct):
                st = state[s]
                eng = nc.vector
                v3 = st["v_ch"][ct][:, 1:L].rearrange("p (y x) -> p y x",
                                                      y=IMG, x=IMG)
                acc = work.tile([128, IMG * IMG], BF16, tag="cacc",
                                bufs=2 * CT, name=f"cacc{ct}")
                acc3 = acc[:].rearrange("p (y x) -> p y x", y=IMG, x=IMG)
                eng.tensor_scalar(out=acc3, in0=v3, scalar1=wdwc[ct][:, 4:5],
                                  scalar2=None, op0=MULT)
                for t in range(9):
                    if t == 4:
                        continue
                    dy, dx = t // 3, t % 3
                    oy0, oy1 = max(0, 1 - dy), IMG + min(0, 1 - dy)
                    ox0, ox1 = max(0, 1 - dx), IMG + min(0, 1 - dx)
                    iy0, ix0 = oy0 + dy - 1, ox0 + dx - 1
                    eng.scalar_tensor_tensor(
                        out=acc3[:, oy0:oy1, ox0:ox1],
                        in0=v3[:, iy0:iy0 + (oy1 - oy0), ix0:ix0 + (ox1 - ox0)],
                        scalar=wdwc[ct][:, t:t + 1],
                        in1=acc3[:, oy0:oy1, ox0:ox1],
                        op0=MULT, op1=ADD)
                st["cacc"][ct] = acc

            def emit_qk(s, m, copy_eng):
                st = state[s]
                p = psum_mm.tile([128, 768], F32, tag="mm")
                for kp in range(CT // 2):
                    w_ap = wqk8[:, 2 * kp:2 * kp + 2, m * 128:(m + 1) * 128]
                    for (n0, nn) in LN_SPLIT:
                        nc.tensor.matmul(
                            p[:, n0:n0 + nn],
                            w_ap,
                            st["x8"][:, 2 * kp:2 * kp + 2, n0:n0 + nn],
                            start=(kp == 0), stop=(kp == CT // 2 - 1),
                            perf_mode=DR)
                dst = work.tile([128, L], BF16, tag="qk", bufs=24,
                                name=f"qk{m}")
                if copy_eng == "act":
                    nc.scalar.copy(dst[:], p[:, 0:L])
                else:
                    nc.vector.tensor_copy(dst[:], p[:, 0:L])
                st["qk"][m] = dst

            def emit_v65(s, ci):
                st = state[s]
                (l0, lp) = L_CHUNKS[ci]
                t = work.tile([128, H * 65], BF16, tag="v65", bufs=2 * NLC,
                              name="v65t")
                t3 = t[:].rearrange("p (h w) -> p h w", h=H, w=65)
                nc.vector.memset(t3[0:lp, :, 64:65], 1.0)
                p = psum_mm.tile([128, 768], F32, tag="mm")
                pb = p[:].bitcast(BF16)
                for ct in range(CT):
                    nc.tensor.transpose(
                        pb[0:lp, ct * 128:(ct + 1) * 128],
                        st["v_ch"][ct][:, l0:l0 + lp],
                        ident[:])
                nc.vector.tensor_copy(
                    t3[0:lp, :, 0:64],
                    pb[0:lp, 0:768].rearrange("p (h w) -> p h w", h=H, w=64))
                st["v65"][ci] = t

            def emit_score_chunk(s, h, ci, expS):
                st = state[s]
                qt = st["qk"][h // 2]
                kt_ = st["qk"][6 + h // 2]
                hb = (h % 2) * 64
                (l0, lp) = L_CHUNKS[ci]
                p = psum_mm.tile([128, 768], F32, tag="mm")
                for (n0, nn) in LN_SPLIT:
                    nc.tensor.matmul(
                        p[0:lp, n0:n0 + nn],
                        kt_[hb:hb + 64, l0:l0 + lp],
                        qt[hb:hb + 64, n0:n0 + nn],
                        start=True, stop=True)
                e = work.tile([128, L], BF16, tag="expS", bufs=8,
                              name="expSt")
                nc.scalar.activation(e[0:lp, :], p[0:lp, 0:L], Exp,
                                     scale=SCALE)
                expS.append(e)

            def emit_pv(s, h, expS):
                st = state[s]
                pv = psum_pv.tile([128, L], F32, tag="pv")
                for ci, (l0, lp) in enumerate(L_CHUNKS):
                    for (n0, nn) in LN_SPLIT:
                        nc.tensor.matmul(
                            pv[0:65, n0:n0 + nn],
                            st["v65"][ci][0:lp, h * 65:(h + 1) * 65],
                            expS[ci][0:lp, n0:n0 + nn],
                            start=(ci == 0), stop=(ci == NLC - 1))
                sums = work.tile([1, L], F32, tag="sums", bufs=4,
                                 name="sums")
                nc.vector.tensor_copy(sums[:], pv[64:65, 0:L])
                rec = work.tile([1, L], F32, tag="rec", bufs=4, name="rec")
                nc.vector.reciprocal_approx_fast(out=rec[:], in_=sums[:])
                st["pv"][h] = pv
                st["rec"][h] = rec

            def emit_norm(s, h):
                st = state[s]
                ct = h // 2
                j = h % 2
                bc = work.tile([64, L], F32, tag="bc", bufs=3, name="bc")
                nc.gpsimd.partition_broadcast(bc[:], st["rec"][h][:])
                nc.vector.tensor_tensor(
                    out=st["attn"][ct][j * 64:(j + 1) * 64, :],
                    in0=st["pv"][h][0:64, :], in1=bc[:], op=MULT)
                st["pv"][h] = None
                if j == 1:
                    # attn[:, 1:] += conv_acc + b_dwc (once per channel tile)
                    nc.vector.scalar_tensor_tensor(
                        out=st["attn"][ct][:, 1:L], in0=st["cacc"][ct][:],
                        scalar=bdwc[ct][:, 0:1],
                        in1=st["attn"][ct][:, 1:L].bitcast(F32),
                        op0=ADD, op1=ADD)

            def emit_proj(s, ci):
                st = state[s]
                (l0, lp) = L_CHUNKS[ci]
                p = psum_mm.tile([128, 768], F32, tag="mm")
                for k in range(CT):
                    for (n0, nn) in ((0, 512), (512, 256)):
                        nc.tensor.matmul(
                            p[0:lp, n0:n0 + nn],
                            st["attn"][k][:, l0:l0 + lp],
                            wprojT[k][:, n0:n0 + nn],
                            start=(k == 0), stop=(k == CT - 1))
                ysb = work.tile([128, C], F32, tag="ysb", bufs=2)
                nc.vector.tensor_tensor(
                    out=ysb[0:lp, :], in0=p[0:lp, :], in1=bproj_bc[0:lp, :],
                    op=ADD)
                nc.sync.dma_start(y_d[s, l0:l0 + lp, :], ysb[0:lp, :])

            def pre_pieces(s):
                """Generator of prep-work pieces for sample s (PE-heavy,
                interleaved into the previous sample's attention)."""
                emit_load(s)
                # first head pair's q,k as early as possible
                emit_qk(s, QK_ORDER[0], "act")
                emit_qk(s, QK_ORDER[1], "act")
                yield
                for m in range(CT):
                    emit_vch(s, m)
                    yield
                for m in QK_ORDER[2:]:
                    emit_qk(s, m, "act")
                    yield
                for ci in range(NLC):
                    emit_v65(s, ci)
                    yield

            def attn_heads(s):
                for h in range(H):
                    if h % 2 == 0:
                        emit_conv(s, h // 2)
                    expS = []
                    for ci in range(NLC):
                        emit_score_chunk(s, h, ci, expS)
                        yield
                    emit_pv(s, h, expS)
                    emit_norm(s, h)
                    yield

            # ---- software-pipelined emission ----
            # warm-up: all prep for sample 0, then the late weight DMAs
            for _ in pre_pieces(0):
                pass
            emit_late_weights()
            for s in range(S):
                interleave = []
                if s + 1 < S:
                    interleave.append(pre_pieces(s + 1))
                if s > 0:
                    def proj_pieces(ps):
                        for ci in range(NLC):
                            emit_proj(ps, ci)
                            yield
                            yield  # spread chunks across twice the rounds
                        del state[ps]
                    interleave.append(proj_pieces(s - 1))
                heads = attn_heads(s)
                # round-robin: one head, then ~3 interleave pieces
                queues = [heads] + interleave
                done = [False] * len(queues)
                weights = [4, 1, 1]  # heads : pre : proj per round
                while not all(done):
                    for qi, q in enumerate(queues):
                        if done[qi]:
                            continue
                        w = weights[qi] if qi < len(weights) else 1
                        for _ in range(w):
                            try:
                                next(q)
                            except StopIteration:
                                done[qi] = True
                                break
            # drain: proj of last sample
            for ci in range(NLC):
                emit_proj(S - 1, ci)

    nc.compile()
    _CACHE[key] = nc
    return nc


def make_in_maps(x, w_qkv, w_proj, b_proj, w_dwc, b_dwc):
    import ml_dtypes
    F8NP = ml_dtypes.float8_e4m3
    x = np.asarray(x, dtype=np.float32)
    N = x.shape[0]
    assert N == N_CORES * S
    # q,k weights, fp8, x32, laid out [p, ct, m] for DoubleRow lhsT
    wqk = np.asarray(w_qkv, np.float32)[:2 * C] * WSCALE        # [2C, C]
    wqk8 = np.ascontiguousarray(
        wqk.T.reshape(CT, 128, 2 * C).transpose(1, 0, 2).astype(F8NP))
    wvT = np.ascontiguousarray(
        np.asarray(w_qkv, np.float32)[2 * C:].T.astype(ml_dtypes.bfloat16))
    wprojT = np.ascontiguousarray(np.asarray(w_proj, np.float32).T)    # [C, C]
    wdwc9 = np.ascontiguousarray(np.asarray(w_dwc, np.float32).reshape(C, 9))
    bdwc = np.ascontiguousarray(np.asarray(b_dwc, np.float32).reshape(C, 1))
    bproj = np.ascontiguousarray(np.asarray(b_proj, np.float32).reshape(1, C))

    in_maps = []
    for i in range(N_CORES):
        xs = x[i * S:(i + 1) * S]                       # [S, L, C]
        xT = np.ascontiguousarray(
            xs.transpose(0, 2, 1).astype(ml_dtypes.bfloat16))  # [S, C, L]
        x8 = np.ascontiguousarray(
            xs.transpose(0, 2, 1).reshape(S, CT, 128, L)
            .transpose(0, 2, 1, 3).astype(F8NP))               # [S, 128, CT, L]
        in_maps.append({"xT": xT, "x8": x8.view(np.uint8),
                        "wqk8": wqk8.view(np.uint8), "wvT": wvT,
                        "wprojT": wprojT, "wdwc": wdwc9, "bdwc": bdwc,
                        "bproj": bproj})
    return in_maps


def kernel(x, w_qkv, w_proj, b_proj, w_dwc, b_dwc):
    global last_results
    nc = _build_nc()
    in_maps = make_in_maps(x, w_qkv, w_proj, b_proj, w_dwc, b_dwc)
    last_results = run_bass_kernel_spmd(nc, in_maps, list(range(N_CORES)))
    y = np.concatenate([r["y"] for r in last_results.results], axis=0)
    return y.astype(np.float32)
